# revision 1
# baseline (speedup 1.0000x reference)
"""Trainium2 Bass kernel for nn_NetworkAction (GNN message passing).

Strategy (8 NeuronCores, agent rows i sharded 128/core):
  conv1 factorizes: h1[i,j,:] = relu(u_i - u_j + b1 + delta_ij*c), u = W1[:, :4] @ s.T.
  Per core: B2 = [-u; -u] stacked [128,1024]; per pair of rows (i0,i1) one ACT op
  builds relu(B2 + [a_i0; a_i1]) into a packed [128,1024] f32r tile.
  conv2 per row: f32r matmul W2T(64x128) x h1(64x512) x2 banks, plus a K=1
  ones-row matmul that accumulates the neighbor mask (0 / -3e38) into PSUM.
  pooled = relu(max_j psum + b2) on DVE/ACT (relu+bias commute with max), then
  max with the constant diagonal contribution h2diag = relu(W2 relu(b1+c) + b2).
  Mask matrix is computed exactly in fp32 elementwise (matches the reference's
  dist<0.5 bit-for-bit); diagonal excluded via a 1e6*eye input slab.
  Head MLP: small fp32 matmuls with K split into <=64 chunks (uniform base per
  PSUM accumulation group; K=128 fp32/f32r matmuls are unsupported by HW).
"""
import sys
sys.path.insert(0, "/opt/trn_rl_repo")
import numpy as np

N = 1024
NCORES = 8
R = N // NCORES  # 128 rows per core

_CACHE = {}


def _build():
    if "nc" in _CACHE:
        return _CACHE["nc"]
    import concourse.bacc as bacc
    import concourse.mybir as mybir
    import concourse.tile as tile

    F32 = mybir.dt.float32
    F32R = mybir.dt.float32r
    AX = mybir.AxisListType
    AF = mybir.ActivationFunctionType
    ALU = mybir.AluOpType

    nc = bacc.Bacc("TRN2", target_bir_lowering=False, debug=False, num_devices=NCORES)

    def din(name, shape):
        return nc.dram_tensor(name, shape, F32, kind="ExternalInput")

    sT_e = din("sT", [4, N])
    xrow_e = din("xrow", [1, N])
    yrow_e = din("yrow", [1, N])
    srows_e = din("srows", [R, 4])
    strows_e = din("strows", [4, R])
    eye_e = din("eye", [R, N])
    w1T4d_e = din("w1T4d", [4, 128])
    b1dup_e = din("b1dup", [128, 1])
    c1w4_e = din("c1w4", [64, 1])
    b1_e = din("b1", [64, 1])
    w2dup_e = din("w2dup", [128, 128])
    b2_e = din("b2", [128, 1])
    fc1aT_e = din("fc1aT", [128, 64])
    fc1bT_e = din("fc1bT", [4, 64])
    fb1_e = din("fb1", [64, 1])
    fc2T_e = din("fc2T", [64, 128])
    fb2_e = din("fb2", [128, 1])
    fc3T_e = din("fc3T", [128, 64])
    fb3_e = din("fb3", [64, 1])
    fc4T_e = din("fc4T", [64, 4])
    fb4_e = din("fb4", [4, 1])
    sperm_e = din("sperm", [4, R])
    gT2_e = din("gT2", [4, R])
    sel4_e = din("sel4", [4, 2])
    out_e = nc.dram_tensor("out", [R, 2], F32, kind="ExternalOutput")

    with tile.TileContext(nc) as tc:
        with (
            tc.tile_pool(name="per", bufs=1) as per,       # persistents
            tc.tile_pool(name="scr", bufs=3) as scr,       # big scratch
            tc.tile_pool(name="h1p", bufs=3) as h1p,       # h1 pair tiles
            tc.tile_pool(name="mrp", bufs=3) as mrp,       # mask-row tiles
        ):
            # ---- load inputs ----
            sT = per.tile([4, N], F32)
            xrow = per.tile([1, N], F32)
            yrow = per.tile([1, N], F32)
            srows = per.tile([R, 4], F32)
            strows = per.tile([4, R], F32)
            eye = per.tile([R, N], F32)
            w1T4d = per.tile([4, 128], F32)
            b1dup = per.tile([128, 1], F32)
            c1w4 = per.tile([64, 1], F32)
            b1 = per.tile([64, 1], F32)
            w2dup = per.tile([128, 128], F32)
            b2 = per.tile([128, 1], F32)
            fc1aT = per.tile([128, 64], F32)
            fc1bT = per.tile([4, 64], F32)
            fb1 = per.tile([64, 1], F32)
            fc2T = per.tile([64, 128], F32)
            fb2 = per.tile([128, 1], F32)
            fc3T = per.tile([128, 64], F32)
            fb3 = per.tile([64, 1], F32)
            fc4T = per.tile([64, 4], F32)
            fb4 = per.tile([4, 1], F32)
            sperm = per.tile([4, R], F32)
            gT2 = per.tile([4, R], F32)
            sel4 = per.tile([4, 2], F32)
            for t, e in ((sT, sT_e), (xrow, xrow_e), (yrow, yrow_e), (srows, srows_e),
                         (strows, strows_e), (eye, eye_e), (w1T4d, w1T4d_e),
                         (b1dup, b1dup_e), (c1w4, c1w4_e), (b1, b1_e), (w2dup, w2dup_e),
                         (b2, b2_e), (fc1aT, fc1aT_e), (fc1bT, fc1bT_e), (fb1, fb1_e),
                         (fc2T, fc2T_e), (fb2, fb2_e), (fc3T, fc3T_e), (fb3, fb3_e),
                         (fc4T, fc4T_e), (fb4, fb4_e), (sperm, sperm_e), (gT2, gT2_e),
                         (sel4, sel4_e)):
                nc.sync.dma_start(t[:], e[:])

            onesf = per.tile([128, 128], F32)
            nc.gpsimd.memset(onesf[:], 1.0)
            ones = per.tile([128, 128], F32R)
            nc.vector.tensor_copy(ones[:], onesf[:])
            w2r = per.tile([128, 128], F32R)
            nc.scalar.copy(w2r[:], w2dup[:])

            # ---- setup: B2, a2, mask slab, h2diag ----
            B2 = per.tile([128, N], F32)
            a2 = per.tile([128, R // 2], F32)
            h2d = per.tile([128, 1], F32)
            slab = per.tile([R, N], F32R)
            pooled = per.tile([128, R], F32)

            with tc.tile_pool(name="pss", bufs=1, space="PSUM") as pss:
                u2 = pss.tile([128, N], F32, tag="u2")
                for bank in range(2):
                    cols = slice(bank * 512, (bank + 1) * 512)
                    nc.tensor.matmul(u2[:, cols], w1T4d[:], sT[:, cols], start=True, stop=True)
                nc.scalar.activation(B2[:], u2[:], AF.Copy, scale=-1.0)

                u2r = pss.tile([128, R], F32, tag="u2r")
                nc.tensor.matmul(u2r[:], w1T4d[:], strows[:], start=True, stop=True)
                a_all = scr.tile([128, R], F32, tag="a_all")
                nc.scalar.activation(a_all[:], u2r[:], AF.Identity, bias=b1dup[:])
                # a2[:, k] = [a_{2k} (top 64) ; a_{2k+1} (bottom 64)]
                a3 = a_all[:].rearrange("p (k two) -> p k two", two=2)
                nc.sync.dma_start(a2[0:64, :], a3[0:64, :, 0])
                nc.sync.dma_start(a2[64:128, :], a3[64:128, :, 1])

                # h2diag = relu(W2 @ relu(c1w4 + b1) + b2)
                h1d = scr.tile([64, 1], F32, tag="h1d")
                nc.scalar.activation(h1d[:], c1w4[:], AF.Relu, bias=b1[:])
                psd = pss.tile([128, 1], F32, tag="psd")
                nc.tensor.matmul(psd[:], w2dup[0:64, :], h1d[:], start=True, stop=True)
                nc.scalar.activation(h2d[:], psd[:], AF.Relu, bias=b2[:])

            # mask slab: slab = (d2 + 1e6*eye >= 0.25) * -3e38, d2 exact fp32
            pbx = scr.tile([R, N], F32, tag="big")
            nc.gpsimd.partition_broadcast(pbx[:], xrow[:])
            pdx = scr.tile([R, N], F32, tag="big")
            nc.vector.tensor_scalar(pdx[:], pbx[:], srows[:, 0:1], None, op0=ALU.subtract)
            sqx = scr.tile([R, N], F32, tag="big")
            nc.scalar.square(sqx[:], pdx[:])
            pby = scr.tile([R, N], F32, tag="big2")
            nc.gpsimd.partition_broadcast(pby[:], yrow[:])
            pdy = scr.tile([R, N], F32, tag="big2")
            nc.vector.tensor_scalar(pdy[:], pby[:], srows[:, 1:2], None, op0=ALU.subtract)
            sqy = scr.tile([R, N], F32, tag="big2")
            nc.scalar.square(sqy[:], pdy[:])
            d2a = scr.tile([R, N], F32, tag="big")
            nc.vector.tensor_tensor(out=d2a[:], in0=sqx[:], in1=sqy[:], op=ALU.add)
            d2p = scr.tile([R, N], F32, tag="big2")
            nc.vector.tensor_tensor(out=d2p[:], in0=d2a[:], in1=eye[:], op=ALU.add)
            nc.vector.tensor_scalar(slab[:], d2p[:], 0.25, -3.0e38, op0=ALU.is_ge, op1=ALU.mult)

            # ---- main loop: 64 pairs ----
            with tc.tile_pool(name="psm", bufs=2, space="PSUM") as psm:
                for k in range(R // 2):
                    h1 = h1p.tile([128, N], F32R, tag="h1")
                    nc.scalar.activation(h1[:], B2[:], AF.Relu, bias=a2[:, k:k + 1])
                    mrow = mrp.tile([65, N], F32R, tag="mr")
                    nc.sync.dma_start(mrow[0:1, :], slab[2 * k:2 * k + 1, :])
                    nc.sync.dma_start(mrow[64:65, :], slab[2 * k + 1:2 * k + 2, :])
                    for ii, pb in ((0, 0), (1, 64)):
                        ps = psm.tile([128, N], F32, tag="ps")
                        for bank in range(2):
                            cols = slice(bank * 512, (bank + 1) * 512)
                            nc.tensor.matmul(ps[:, cols], w2r[pb:pb + 64, :],
                                             h1[pb:pb + 64, cols], start=True, stop=False)
                            nc.tensor.matmul(ps[:, cols], ones[pb:pb + 1, :],
                                             mrow[pb:pb + 1, cols], start=False, stop=True)
                        idx = 2 * k + ii
                        nc.vector.reduce_max(pooled[:, idx:idx + 1], ps[:], axis=AX.X)

            # ---- finalize pooled + head MLP ----
            with tc.tile_pool(name="psh", bufs=1, space="PSUM") as psh:
                poolr = scr.tile([128, R], F32, tag="poolr")
                nc.scalar.activation(poolr[:], pooled[:], AF.Relu, bias=b2[:])
                poolF = scr.tile([128, R], F32, tag="poolF")
                nc.vector.tensor_scalar(poolF[:], poolr[:], h2d[:], None, op0=ALU.max)

                sgv2 = scr.tile([4, R], F32, tag="sgv2")
                nc.vector.tensor_tensor(out=sgv2[:], in0=sperm[:], in1=gT2[:], op=ALU.subtract)

                # fc1: K=132 split 64+64+4 (uniform base per group; cross-psum for base 64)
                z1a = psh.tile([64, R], F32, tag="z1a")
                nc.tensor.matmul(z1a[:], fc1aT[0:64, :], poolF[0:64, :], start=True, stop=False)
                nc.tensor.matmul(z1a[:], fc1bT[:], sgv2[:], start=False, stop=True)
                z1b = psh.tile([64, R], F32, tag="z1b")
                nc.tensor.matmul(z1b[:], fc1aT[64:128, :], poolF[64:128, :], start=True, stop=True)
                z1s = scr.tile([64, R], F32, tag="z1s")
                nc.scalar.copy(z1s[:], z1b[:])
                z1pre = scr.tile([64, R], F32, tag="z1pre")
                nc.vector.tensor_tensor(out=z1pre[:], in0=z1a[:], in1=z1s[:], op=ALU.add)
                z1 = scr.tile([64, R], F32, tag="z1")
                nc.scalar.activation(z1[:], z1pre[:], AF.Relu, bias=fb1[:])

                z2p = psh.tile([128, R], F32, tag="z2p")
                nc.tensor.matmul(z2p[:], fc2T[:], z1[:], start=True, stop=True)
                z2 = scr.tile([128, R], F32, tag="z2")
                nc.scalar.activation(z2[:], z2p[:], AF.Relu, bias=fb2[:])

                z3a = psh.tile([64, R], F32, tag="z3a")
                nc.tensor.matmul(z3a[:], fc3T[0:64, :], z2[0:64, :], start=True, stop=True)
                z3b = psh.tile([64, R], F32, tag="z3b")
                nc.tensor.matmul(z3b[:], fc3T[64:128, :], z2[64:128, :], start=True, stop=True)
                z3s = scr.tile([64, R], F32, tag="z3s")
                nc.scalar.copy(z3s[:], z3b[:])
                z3pre = scr.tile([64, R], F32, tag="z3pre")
                nc.vector.tensor_tensor(out=z3pre[:], in0=z3a[:], in1=z3s[:], op=ALU.add)
                z3 = scr.tile([64, R], F32, tag="z3")
                nc.scalar.activation(z3[:], z3pre[:], AF.Relu, bias=fb3[:])

                kp = psh.tile([4, R], F32, tag="kp")
                nc.tensor.matmul(kp[:], fc4T[:], z3[:], start=True, stop=True)
                ksig = scr.tile([4, R], F32, tag="ksig")
                nc.scalar.activation(ksig[:], kp[:], AF.Sigmoid, bias=fb4[:])
                k2n = scr.tile([4, R], F32, tag="k2n")
                nc.vector.tensor_scalar(k2n[:], ksig[:], -2.0, 1.0, op0=ALU.mult, op1=ALU.add)
                P = scr.tile([4, R], F32, tag="P")
                nc.vector.tensor_tensor(out=P[:], in0=k2n[:], in1=sgv2[:], op=ALU.mult)
                av = psh.tile([2, R], F32, tag="av")
                nc.tensor.matmul(av[:], sel4[:], P[:], start=True, stop=True)
                res = scr.tile([2, R], F32, tag="res")
                nc.scalar.copy(res[:], av[:])
                nc.sync.dma_start(out_e[:].rearrange("i o -> o i"), res[:])

    nc.compile()
    _CACHE["nc"] = nc
    return nc


def _in_maps(inputs):
    f = np.float32
    s = np.asarray(inputs["s"], f)
    g = np.asarray(inputs["g"], f)
    c1w = np.asarray(inputs["conv1_w"], f)
    c1b = np.asarray(inputs["conv1_b"], f)
    c2w = np.asarray(inputs["conv2_w"], f)
    c2b = np.asarray(inputs["conv2_b"], f)
    f1w = np.asarray(inputs["fc1_w"], f)
    f1b = np.asarray(inputs["fc1_b"], f)
    f2w = np.asarray(inputs["fc2_w"], f)
    f2b = np.asarray(inputs["fc2_b"], f)
    f3w = np.asarray(inputs["fc3_w"], f)
    f3b = np.asarray(inputs["fc3_b"], f)
    f4w = np.asarray(inputs["fc4_w"], f)
    f4b = np.asarray(inputs["fc4_b"], f)

    sT = np.ascontiguousarray(s.T)
    shared = {
        "sT": sT,
        "xrow": np.ascontiguousarray(sT[0:1]),
        "yrow": np.ascontiguousarray(sT[1:2]),
        "w1T4d": np.ascontiguousarray(np.tile(c1w[:, :4].T, (1, 2))),
        "b1dup": np.tile(c1b, 2)[:, None].astype(f),
        "c1w4": np.ascontiguousarray(c1w[:, 4:5]),
        "b1": c1b[:, None].copy(),
        "w2dup": np.ascontiguousarray(np.tile(c2w.T, (2, 1))),
        "b2": c2b[:, None].copy(),
        "fc1aT": np.ascontiguousarray(f1w[:, :128].T),
        "fc1bT": np.ascontiguousarray(f1w[:, [128, 130, 129, 131]].T),
        "fb1": f1b[:, None].copy(),
        "fc2T": np.ascontiguousarray(f2w.T),
        "fb2": f2b[:, None].copy(),
        "fc3T": np.ascontiguousarray(f3w.T),
        "fb3": f3b[:, None].copy(),
        "fc4T": np.ascontiguousarray(f4w.T),
        "fb4": f4b[:, None].copy(),
        "sel4": np.array([[1, 0], [1, 0], [0, 1], [0, 1]], f),
    }
    eyeN = np.eye(N, dtype=f) * f(1e6)
    maps = []
    for c in range(NCORES):
        r0 = c * R
        rows = slice(r0, r0 + R)
        m = dict(shared)
        m["srows"] = s[rows].copy()
        m["strows"] = np.ascontiguousarray(s[rows].T)
        m["eye"] = eyeN[rows].copy()
        m["sperm"] = np.ascontiguousarray(s[rows][:, [0, 2, 1, 3]].T)
        gT2 = np.zeros((4, R), f)
        gT2[0] = g[rows, 0]
        gT2[2] = g[rows, 1]
        m["gT2"] = gT2
        maps.append(m)
    return maps


def kernel(**inputs) -> np.ndarray:
    from concourse.bass_utils import run_bass_kernel_spmd
    nc = _build()
    res = run_bass_kernel_spmd(nc, _in_maps(inputs), list(range(NCORES)))
    return np.concatenate([res.results[c]["out"] for c in range(NCORES)], axis=0)


# revision 2
# speedup vs baseline: 1.2199x; 1.2199x over previous
"""Trainium2 Bass kernel for nn_NetworkAction (GNN message passing).

Strategy (8 NeuronCores, agent rows i sharded 128/core):
  conv1 factorizes: h1[i,j,:] = relu(u_i - u_j + b1 + delta_ij*c), u = W1[:, :4] @ s.T.
  Per core: B2 = [-u; -u] stacked [128,1024]; per pair of rows (i0,i1) one ACT op
  builds relu(B2 + [a_i0; a_i1]) into a packed [128,1024] f32r tile.
  conv2 per row: f32r matmul W2T(64x128) x h1(64x512) x2 banks, plus a K=1
  ones-row matmul that accumulates the neighbor mask (0 / -3e38) into PSUM.
  pooled = relu(max_j psum + b2) on DVE/ACT (relu+bias commute with max), then
  max with the constant diagonal contribution h2diag = relu(W2 relu(b1+c) + b2).
  Mask matrix is computed exactly in fp32 elementwise (matches the reference's
  dist<0.5 bit-for-bit); diagonal excluded via a 1e6*eye input slab.
  Head MLP: small fp32 matmuls with K split into <=64 chunks (uniform base per
  PSUM accumulation group; K=128 fp32/f32r matmuls are unsupported by HW).
"""
import sys
sys.path.insert(0, "/opt/trn_rl_repo")
import numpy as np

N = 1024
NCORES = 8
R = N // NCORES  # 128 rows per core

_CACHE = {}


def _build():
    if "nc" in _CACHE:
        return _CACHE["nc"]
    import concourse.bacc as bacc
    import concourse.mybir as mybir
    import concourse.tile as tile

    F32 = mybir.dt.float32
    F32R = mybir.dt.float32r
    AX = mybir.AxisListType
    AF = mybir.ActivationFunctionType
    ALU = mybir.AluOpType

    nc = bacc.Bacc("TRN2", target_bir_lowering=False, debug=False, num_devices=NCORES)

    def din(name, shape):
        return nc.dram_tensor(name, shape, F32, kind="ExternalInput")

    sT_e = din("sT", [4, N])
    xrow_e = din("xrow", [1, N])
    yrow_e = din("yrow", [1, N])
    srows_e = din("srows", [R, 4])
    strows_e = din("strows", [4, R])
    eye_e = din("eye", [R, N])
    w1T4d_e = din("w1T4d", [4, 128])
    b1dup_e = din("b1dup", [128, 1])
    c1w4_e = din("c1w4", [64, 1])
    b1_e = din("b1", [64, 1])
    w2dup_e = din("w2dup", [128, 128])
    b2_e = din("b2", [128, 1])
    fc1aT_e = din("fc1aT", [128, 64])
    fc1bT_e = din("fc1bT", [4, 64])
    fb1_e = din("fb1", [64, 1])
    fc2T_e = din("fc2T", [64, 128])
    fb2_e = din("fb2", [128, 1])
    fc3T_e = din("fc3T", [128, 64])
    fb3_e = din("fb3", [64, 1])
    fc4T_e = din("fc4T", [64, 4])
    fb4_e = din("fb4", [4, 1])
    sperm_e = din("sperm", [4, R])
    gT2_e = din("gT2", [4, R])
    sel4_e = din("sel4", [4, 2])
    out_e = nc.dram_tensor("out", [R, 2], F32, kind="ExternalOutput")

    with tile.TileContext(nc) as tc:
        with (
            tc.tile_pool(name="per", bufs=1) as per,       # persistents
            tc.tile_pool(name="scr", bufs=3) as scr,       # big scratch
            tc.tile_pool(name="h1p", bufs=3) as h1p,       # h1 pair tiles
            tc.tile_pool(name="mrp", bufs=3) as mrp,       # mask-row tiles
        ):
            # ---- load inputs ----
            sT = per.tile([4, N], F32)
            xrow = per.tile([1, N], F32)
            yrow = per.tile([1, N], F32)
            srows = per.tile([R, 4], F32)
            strows = per.tile([4, R], F32)
            eye = per.tile([R, N], F32)
            w1T4d = per.tile([4, 128], F32)
            b1dup = per.tile([128, 1], F32)
            c1w4 = per.tile([64, 1], F32)
            b1 = per.tile([64, 1], F32)
            w2dup = per.tile([128, 128], F32)
            b2 = per.tile([128, 1], F32)
            fc1aT = per.tile([128, 64], F32)
            fc1bT = per.tile([4, 64], F32)
            fb1 = per.tile([64, 1], F32)
            fc2T = per.tile([64, 128], F32)
            fb2 = per.tile([128, 1], F32)
            fc3T = per.tile([128, 64], F32)
            fb3 = per.tile([64, 1], F32)
            fc4T = per.tile([64, 4], F32)
            fb4 = per.tile([4, 1], F32)
            sperm = per.tile([4, R], F32)
            gT2 = per.tile([4, R], F32)
            sel4 = per.tile([4, 2], F32)
            for t, e in ((sT, sT_e), (xrow, xrow_e), (yrow, yrow_e), (srows, srows_e),
                         (strows, strows_e), (eye, eye_e), (w1T4d, w1T4d_e),
                         (b1dup, b1dup_e), (c1w4, c1w4_e), (b1, b1_e), (w2dup, w2dup_e),
                         (b2, b2_e), (fc1aT, fc1aT_e), (fc1bT, fc1bT_e), (fb1, fb1_e),
                         (fc2T, fc2T_e), (fb2, fb2_e), (fc3T, fc3T_e), (fb3, fb3_e),
                         (fc4T, fc4T_e), (fb4, fb4_e), (sperm, sperm_e), (gT2, gT2_e),
                         (sel4, sel4_e)):
                nc.sync.dma_start(t[:], e[:])

            onesf = per.tile([128, 128], F32)
            nc.gpsimd.memset(onesf[:], 1.0)
            ones = per.tile([128, 128], F32R)
            nc.vector.tensor_copy(ones[:], onesf[:])
            w2r = per.tile([128, 128], F32R)
            nc.scalar.copy(w2r[:], w2dup[:])

            # ---- setup: B2, a2, mask slab, h2diag ----
            B2 = per.tile([128, N], F32)
            a2 = per.tile([128, R // 2], F32)
            h2d = per.tile([128, 1], F32)
            slab = per.tile([R, N], F32R)
            pooled = per.tile([128, R], F32)

            with tc.tile_pool(name="pss", bufs=1, space="PSUM") as pss:
                u2 = pss.tile([128, N], F32, tag="u2")
                for bank in range(2):
                    cols = slice(bank * 512, (bank + 1) * 512)
                    nc.tensor.matmul(u2[:, cols], w1T4d[:], sT[:, cols], start=True, stop=True)
                nc.scalar.activation(B2[:], u2[:], AF.Copy, scale=-1.0)

                u2r = pss.tile([128, R], F32, tag="u2r")
                nc.tensor.matmul(u2r[:], w1T4d[:], strows[:], start=True, stop=True)
                a_all = scr.tile([128, R], F32, tag="a_all")
                nc.scalar.activation(a_all[:], u2r[:], AF.Identity, bias=b1dup[:])
                # a2[:, k] = [a_{2k} (top 64) ; a_{2k+1} (bottom 64)]
                a3 = a_all[:].rearrange("p (k two) -> p k two", two=2)
                nc.sync.dma_start(a2[0:64, :], a3[0:64, :, 0])
                nc.sync.dma_start(a2[64:128, :], a3[64:128, :, 1])

                # h2diag = relu(W2 @ relu(c1w4 + b1) + b2)
                h1d = scr.tile([64, 1], F32, tag="h1d")
                nc.scalar.activation(h1d[:], c1w4[:], AF.Relu, bias=b1[:])
                psd = pss.tile([128, 1], F32, tag="psd")
                nc.tensor.matmul(psd[:], w2dup[0:64, :], h1d[:], start=True, stop=True)
                nc.scalar.activation(h2d[:], psd[:], AF.Relu, bias=b2[:])

            # mask slab: slab = (d2 + 1e6*eye >= 0.25) * -3e38, d2 exact fp32
            pbx = scr.tile([R, N], F32, tag="big")
            nc.gpsimd.partition_broadcast(pbx[:], xrow[:])
            pdx = scr.tile([R, N], F32, tag="big")
            nc.vector.tensor_scalar(pdx[:], pbx[:], srows[:, 0:1], None, op0=ALU.subtract)
            sqx = scr.tile([R, N], F32, tag="big")
            nc.scalar.square(sqx[:], pdx[:])
            pby = scr.tile([R, N], F32, tag="big2")
            nc.gpsimd.partition_broadcast(pby[:], yrow[:])
            pdy = scr.tile([R, N], F32, tag="big2")
            nc.vector.tensor_scalar(pdy[:], pby[:], srows[:, 1:2], None, op0=ALU.subtract)
            sqy = scr.tile([R, N], F32, tag="big2")
            nc.scalar.square(sqy[:], pdy[:])
            d2a = scr.tile([R, N], F32, tag="big")
            nc.vector.tensor_tensor(out=d2a[:], in0=sqx[:], in1=sqy[:], op=ALU.add)
            d2p = scr.tile([R, N], F32, tag="big2")
            nc.vector.tensor_tensor(out=d2p[:], in0=d2a[:], in1=eye[:], op=ALU.add)
            nc.vector.tensor_scalar(slab[:], d2p[:], 0.25, -3.0e38, op0=ALU.is_ge, op1=ALU.mult)

            # ---- main loop: 64 pairs ----
            # PE order groups same-weight matmuls (w2r top, ones top, w2r bot,
            # ones bot) so LDWEIGHTS amortizes over 2 matmuls instead of
            # alternating every instruction.
            with tc.tile_pool(name="psm", bufs=4, space="PSUM") as psm:
                for k in range(R // 2):
                    h1 = h1p.tile([128, N], F32R, tag="h1")
                    nc.scalar.activation(h1[:], B2[:], AF.Relu, bias=a2[:, k:k + 1])
                    mrow = mrp.tile([65, N], F32R, tag="mr")
                    nc.sync.dma_start(mrow[0:1, :], slab[2 * k:2 * k + 1, :])
                    nc.sync.dma_start(mrow[64:65, :], slab[2 * k + 1:2 * k + 2, :])
                    pstile = {}
                    for ii, pb in ((0, 0), (1, 64)):
                        ps = psm.tile([128, N], F32, tag="ps")
                        pstile[ii] = ps
                        for bank in range(2):
                            cols = slice(bank * 512, (bank + 1) * 512)
                            nc.tensor.matmul(ps[:, cols], w2r[pb:pb + 64, :],
                                             h1[pb:pb + 64, cols], start=True, stop=False)
                        for bank in range(2):
                            cols = slice(bank * 512, (bank + 1) * 512)
                            nc.tensor.matmul(ps[:, cols], ones[pb:pb + 1, :],
                                             mrow[pb:pb + 1, cols], start=False, stop=True)
                    for ii in (0, 1):
                        idx = 2 * k + ii
                        nc.vector.reduce_max(pooled[:, idx:idx + 1], pstile[ii][:], axis=AX.X)

            # ---- finalize pooled + head MLP ----
            with tc.tile_pool(name="psh", bufs=1, space="PSUM") as psh:
                poolr = scr.tile([128, R], F32, tag="poolr")
                nc.scalar.activation(poolr[:], pooled[:], AF.Relu, bias=b2[:])
                poolF = scr.tile([128, R], F32, tag="poolF")
                nc.vector.tensor_scalar(poolF[:], poolr[:], h2d[:], None, op0=ALU.max)

                sgv2 = scr.tile([4, R], F32, tag="sgv2")
                nc.vector.tensor_tensor(out=sgv2[:], in0=sperm[:], in1=gT2[:], op=ALU.subtract)

                # fc1: K=132 split 64+64+4 (uniform base per group; cross-psum for base 64)
                z1a = psh.tile([64, R], F32, tag="z1a")
                nc.tensor.matmul(z1a[:], fc1aT[0:64, :], poolF[0:64, :], start=True, stop=False)
                nc.tensor.matmul(z1a[:], fc1bT[:], sgv2[:], start=False, stop=True)
                z1b = psh.tile([64, R], F32, tag="z1b")
                nc.tensor.matmul(z1b[:], fc1aT[64:128, :], poolF[64:128, :], start=True, stop=True)
                z1s = scr.tile([64, R], F32, tag="z1s")
                nc.scalar.copy(z1s[:], z1b[:])
                z1pre = scr.tile([64, R], F32, tag="z1pre")
                nc.vector.tensor_tensor(out=z1pre[:], in0=z1a[:], in1=z1s[:], op=ALU.add)
                z1 = scr.tile([64, R], F32, tag="z1")
                nc.scalar.activation(z1[:], z1pre[:], AF.Relu, bias=fb1[:])

                z2p = psh.tile([128, R], F32, tag="z2p")
                nc.tensor.matmul(z2p[:], fc2T[:], z1[:], start=True, stop=True)
                z2 = scr.tile([128, R], F32, tag="z2")
                nc.scalar.activation(z2[:], z2p[:], AF.Relu, bias=fb2[:])

                z3a = psh.tile([64, R], F32, tag="z3a")
                nc.tensor.matmul(z3a[:], fc3T[0:64, :], z2[0:64, :], start=True, stop=True)
                z3b = psh.tile([64, R], F32, tag="z3b")
                nc.tensor.matmul(z3b[:], fc3T[64:128, :], z2[64:128, :], start=True, stop=True)
                z3s = scr.tile([64, R], F32, tag="z3s")
                nc.scalar.copy(z3s[:], z3b[:])
                z3pre = scr.tile([64, R], F32, tag="z3pre")
                nc.vector.tensor_tensor(out=z3pre[:], in0=z3a[:], in1=z3s[:], op=ALU.add)
                z3 = scr.tile([64, R], F32, tag="z3")
                nc.scalar.activation(z3[:], z3pre[:], AF.Relu, bias=fb3[:])

                kp = psh.tile([4, R], F32, tag="kp")
                nc.tensor.matmul(kp[:], fc4T[:], z3[:], start=True, stop=True)
                ksig = scr.tile([4, R], F32, tag="ksig")
                nc.scalar.activation(ksig[:], kp[:], AF.Sigmoid, bias=fb4[:])
                k2n = scr.tile([4, R], F32, tag="k2n")
                nc.vector.tensor_scalar(k2n[:], ksig[:], -2.0, 1.0, op0=ALU.mult, op1=ALU.add)
                P = scr.tile([4, R], F32, tag="P")
                nc.vector.tensor_tensor(out=P[:], in0=k2n[:], in1=sgv2[:], op=ALU.mult)
                av = psh.tile([2, R], F32, tag="av")
                nc.tensor.matmul(av[:], sel4[:], P[:], start=True, stop=True)
                res = scr.tile([2, R], F32, tag="res")
                nc.scalar.copy(res[:], av[:])
                nc.sync.dma_start(out_e[:].rearrange("i o -> o i"), res[:])

    nc.compile()
    _CACHE["nc"] = nc
    return nc


def _in_maps(inputs):
    f = np.float32
    s = np.asarray(inputs["s"], f)
    g = np.asarray(inputs["g"], f)
    c1w = np.asarray(inputs["conv1_w"], f)
    c1b = np.asarray(inputs["conv1_b"], f)
    c2w = np.asarray(inputs["conv2_w"], f)
    c2b = np.asarray(inputs["conv2_b"], f)
    f1w = np.asarray(inputs["fc1_w"], f)
    f1b = np.asarray(inputs["fc1_b"], f)
    f2w = np.asarray(inputs["fc2_w"], f)
    f2b = np.asarray(inputs["fc2_b"], f)
    f3w = np.asarray(inputs["fc3_w"], f)
    f3b = np.asarray(inputs["fc3_b"], f)
    f4w = np.asarray(inputs["fc4_w"], f)
    f4b = np.asarray(inputs["fc4_b"], f)

    sT = np.ascontiguousarray(s.T)
    shared = {
        "sT": sT,
        "xrow": np.ascontiguousarray(sT[0:1]),
        "yrow": np.ascontiguousarray(sT[1:2]),
        "w1T4d": np.ascontiguousarray(np.tile(c1w[:, :4].T, (1, 2))),
        "b1dup": np.tile(c1b, 2)[:, None].astype(f),
        "c1w4": np.ascontiguousarray(c1w[:, 4:5]),
        "b1": c1b[:, None].copy(),
        "w2dup": np.ascontiguousarray(np.tile(c2w.T, (2, 1))),
        "b2": c2b[:, None].copy(),
        "fc1aT": np.ascontiguousarray(f1w[:, :128].T),
        "fc1bT": np.ascontiguousarray(f1w[:, [128, 130, 129, 131]].T),
        "fb1": f1b[:, None].copy(),
        "fc2T": np.ascontiguousarray(f2w.T),
        "fb2": f2b[:, None].copy(),
        "fc3T": np.ascontiguousarray(f3w.T),
        "fb3": f3b[:, None].copy(),
        "fc4T": np.ascontiguousarray(f4w.T),
        "fb4": f4b[:, None].copy(),
        "sel4": np.array([[1, 0], [1, 0], [0, 1], [0, 1]], f),
    }
    eyeN = np.eye(N, dtype=f) * f(1e6)
    maps = []
    for c in range(NCORES):
        r0 = c * R
        rows = slice(r0, r0 + R)
        m = dict(shared)
        m["srows"] = s[rows].copy()
        m["strows"] = np.ascontiguousarray(s[rows].T)
        m["eye"] = eyeN[rows].copy()
        m["sperm"] = np.ascontiguousarray(s[rows][:, [0, 2, 1, 3]].T)
        gT2 = np.zeros((4, R), f)
        gT2[0] = g[rows, 0]
        gT2[2] = g[rows, 1]
        m["gT2"] = gT2
        maps.append(m)
    return maps


def kernel(**inputs) -> np.ndarray:
    from concourse.bass_utils import run_bass_kernel_spmd
    nc = _build()
    res = run_bass_kernel_spmd(nc, _in_maps(inputs), list(range(NCORES)))
    return np.concatenate([res.results[c]["out"] for c in range(NCORES)], axis=0)


# revision 11
# speedup vs baseline: 2.1147x; 1.7335x over previous
"""Trainium2 Bass kernel for nn_NetworkAction (GNN message passing).

Strategy (8 NeuronCores, agent rows i sharded 128/core):
  conv1 factorizes: h1[i,j,:] = relu(u_i - u_j + b1 + delta_ij*c), u = W1[:, :4] @ s.T.
  Per core: B2 = [-u; -u] stacked [128,1024]; per pair of rows (i0,i1) one ACT op
  builds relu(B2 + [a_i0; a_i1]) into a packed [128,1024] f32r tile.
  conv2 per row: f32r matmul W2T(64x128) x h1(64x512) x2 banks, plus a K=1
  ones-row matmul that accumulates the neighbor mask (0 / -3e38) into PSUM.
  pooled = relu(max_j psum + b2) on DVE/ACT (relu+bias commute with max), then
  max with the constant diagonal contribution h2diag = relu(W2 relu(b1+c) + b2).
  Mask matrix is computed exactly in fp32 elementwise (matches the reference's
  dist<0.5 bit-for-bit); diagonal excluded via a 1e6*eye input slab.
  Head MLP: small fp32 matmuls with K split into <=64 chunks (uniform base per
  PSUM accumulation group; K=128 fp32/f32r matmuls are unsupported by HW).
"""
import os
import sys
sys.path.insert(0, "/opt/trn_rl_repo")
import numpy as np

N = 1024
NCORES = 8
R = N // NCORES  # 128 rows per core
USE_BF16 = os.environ.get("KERNEL_F32R", "") == ""  # bf16 K=65 path by default

_CACHE = {}


def _build():
    if "nc" in _CACHE:
        return _CACHE["nc"]
    import concourse.bacc as bacc
    import concourse.mybir as mybir
    import concourse.tile as tile

    F32 = mybir.dt.float32
    F32R = mybir.dt.float32r
    AX = mybir.AxisListType
    AF = mybir.ActivationFunctionType
    ALU = mybir.AluOpType

    nc = bacc.Bacc("TRN2", target_bir_lowering=False, debug=False, num_devices=NCORES)

    def din(name, shape):
        return nc.dram_tensor(name, shape, F32, kind="ExternalInput")

    sT_e = din("sT", [4, N])
    xrow_e = din("xrow", [1, N])
    yrow_e = din("yrow", [1, N])
    srows_e = din("srows", [R, 4])
    strows_e = din("strows", [4, R])
    eye_e = din("eye", [R, N])
    w1T4d_e = din("w1T4d", [4, 128])
    b1dup_e = din("b1dup", [128, 1])
    c1w4_e = din("c1w4", [64, 1])
    b1_e = din("b1", [64, 1])
    w2dup_e = din("w2dup", [128, 128])
    w2K65_e = din("w2K65", [65, 128])
    b2_e = din("b2", [128, 1])
    fc1aT_e = din("fc1aT", [128, 64])
    fc1bT_e = din("fc1bT", [4, 64])
    fb1_e = din("fb1", [64, 1])
    fc2T_e = din("fc2T", [64, 128])
    fb2_e = din("fb2", [128, 1])
    fc3T_e = din("fc3T", [128, 64])
    fb3_e = din("fb3", [64, 1])
    fc4T_e = din("fc4T", [64, 4])
    fb4_e = din("fb4", [4, 1])
    sperm_e = din("sperm", [4, R])
    gT2_e = din("gT2", [4, R])
    sel4_e = din("sel4", [4, 2])
    out_e = nc.dram_tensor("out", [R, 2], F32, kind="ExternalOutput")

    with tile.TileContext(nc) as tc:
        with (
            tc.tile_pool(name="per", bufs=1) as per,       # persistents
            tc.tile_pool(name="scr", bufs=3) as scr,       # big scratch
            tc.tile_pool(name="h1p", bufs=3) as h1p,       # h1 pair tiles
            tc.tile_pool(name="mrp", bufs=3) as mrp,       # mask-row tiles
        ):
            # ---- load inputs ----
            sT = per.tile([4, N], F32)
            xrow = per.tile([1, N], F32)
            yrow = per.tile([1, N], F32)
            srows = per.tile([R, 4], F32)
            strows = per.tile([4, R], F32)
            eye = per.tile([R, N], F32)
            w1T4d = per.tile([4, 128], F32)
            b1dup = per.tile([128, 1], F32)
            c1w4 = per.tile([64, 1], F32)
            b1 = per.tile([64, 1], F32)
            w2dup = per.tile([128, 128], F32)
            w2K65 = per.tile([65, 128], F32)
            b2 = per.tile([128, 1], F32)
            fc1aT = per.tile([128, 64], F32)
            fc1bT = per.tile([4, 64], F32)
            fb1 = per.tile([64, 1], F32)
            fc2T = per.tile([64, 128], F32)
            fb2 = per.tile([128, 1], F32)
            fc3T = per.tile([128, 64], F32)
            fb3 = per.tile([64, 1], F32)
            fc4T = per.tile([64, 4], F32)
            fb4 = per.tile([4, 1], F32)
            sperm = per.tile([4, R], F32)
            gT2 = per.tile([4, R], F32)
            sel4 = per.tile([4, 2], F32)
            for t, e in ((sT, sT_e), (xrow, xrow_e), (yrow, yrow_e), (srows, srows_e),
                         (strows, strows_e), (eye, eye_e), (w1T4d, w1T4d_e),
                         (b1dup, b1dup_e), (c1w4, c1w4_e), (b1, b1_e), (w2dup, w2dup_e),
                         (w2K65, w2K65_e),
                         (b2, b2_e), (fc1aT, fc1aT_e), (fc1bT, fc1bT_e), (fb1, fb1_e),
                         (fc2T, fc2T_e), (fb2, fb2_e), (fc3T, fc3T_e), (fb3, fb3_e),
                         (fc4T, fc4T_e), (fb4, fb4_e), (sperm, sperm_e), (gT2, gT2_e),
                         (sel4, sel4_e)):
                nc.sync.dma_start(t[:], e[:])

            BF16 = mybir.dt.bfloat16
            MDT = BF16 if USE_BF16 else F32R
            if USE_BF16:
                w2b = per.tile([65, 128], BF16)
                nc.scalar.copy(w2b[:], w2K65[:])
            else:
                onesf = per.tile([128, 128], F32)
                nc.gpsimd.memset(onesf[:], 1.0)
                ones = per.tile([128, 128], F32R)
                nc.vector.tensor_copy(ones[:], onesf[:])
                w2r = per.tile([128, 128], F32R)
                nc.scalar.copy(w2r[:], w2dup[:])

            # ---- setup: B2, a2, mask slab, h2diag ----
            B2 = per.tile([128, N], F32)
            a2 = per.tile([128, R // 2], F32)
            h2d = per.tile([128, 1], F32)
            slab = per.tile([R, N], MDT)
            pooled = per.tile([128, R], F32)

            with tc.tile_pool(name="pss", bufs=1, space="PSUM") as pss:
                u2 = pss.tile([128, N], F32, tag="u2")
                for bank in range(2):
                    cols = slice(bank * 512, (bank + 1) * 512)
                    nc.tensor.matmul(u2[:, cols], w1T4d[:], sT[:, cols], start=True, stop=True)
                nc.scalar.activation(B2[:], u2[:], AF.Copy, scale=-1.0)

                u2r = pss.tile([128, R], F32, tag="u2r")
                nc.tensor.matmul(u2r[:], w1T4d[:], strows[:], start=True, stop=True)
                a_all = scr.tile([128, R], F32, tag="a_all")
                nc.scalar.activation(a_all[:], u2r[:], AF.Identity, bias=b1dup[:])
                # a2[:, k] = [a_{2k} (top 64) ; a_{2k+1} (bottom 64)]
                a3 = a_all[:].rearrange("p (k two) -> p k two", two=2)
                nc.sync.dma_start(a2[0:64, :], a3[0:64, :, 0])
                nc.sync.dma_start(a2[64:128, :], a3[64:128, :, 1])

                # h2diag = relu(W2 @ relu(c1w4 + b1) + b2)
                h1d = scr.tile([64, 1], F32, tag="h1d")
                nc.scalar.activation(h1d[:], c1w4[:], AF.Relu, bias=b1[:])
                psd = pss.tile([128, 1], F32, tag="psd")
                nc.tensor.matmul(psd[:], w2dup[0:64, :], h1d[:], start=True, stop=True)
                nc.scalar.activation(h2d[:], psd[:], AF.Relu, bias=b2[:])

            # mask slab: slab = (d2 + 1e6*eye >= 0.25) * -3e38, d2 exact fp32
            pbx = scr.tile([R, N], F32, tag="big")
            nc.gpsimd.partition_broadcast(pbx[:], xrow[:])
            pdx = scr.tile([R, N], F32, tag="big")
            nc.vector.tensor_scalar(pdx[:], pbx[:], srows[:, 0:1], None, op0=ALU.subtract)
            sqx = scr.tile([R, N], F32, tag="big")
            nc.scalar.square(sqx[:], pdx[:])
            pby = scr.tile([R, N], F32, tag="big2")
            nc.gpsimd.partition_broadcast(pby[:], yrow[:])
            pdy = scr.tile([R, N], F32, tag="big2")
            nc.vector.tensor_scalar(pdy[:], pby[:], srows[:, 1:2], None, op0=ALU.subtract)
            sqy = scr.tile([R, N], F32, tag="big2")
            nc.scalar.square(sqy[:], pdy[:])
            d2a = scr.tile([R, N], F32, tag="big")
            nc.vector.tensor_tensor(out=d2a[:], in0=sqx[:], in1=sqy[:], op=ALU.add)
            d2p = scr.tile([R, N], F32, tag="big2")
            nc.vector.tensor_tensor(out=d2p[:], in0=d2a[:], in1=eye[:], op=ALU.add)
            nc.vector.tensor_scalar(slab[:], d2p[:], 0.25, -3.0e38, op0=ALU.is_ge, op1=ALU.mult)

            # ---- main loop ----
            with tc.tile_pool(name="psm", bufs=4, space="PSUM") as psm:
                if USE_BF16:
                    # per row i: h1t [65, N] bf16 = [relu(B + a_i); maskrow_i],
                    # one K=65 matmul per 512-col bank (mask rides contraction).
                    for i in range(R):
                        h1t = h1p.tile([65, N], BF16, tag="h1")
                        nc.scalar.activation(h1t[0:64, :], B2[0:64, :], AF.Relu,
                                             bias=a_all[0:64, i:i + 1])
                        nc.sync.dma_start(h1t[64:65, :], slab[i:i + 1, :])
                        ps = psm.tile([128, N], F32, tag="ps")
                        for bank in range(2):
                            cols = slice(bank * 512, (bank + 1) * 512)
                            nc.tensor.matmul(ps[:, cols], w2b[:], h1t[:, cols],
                                             start=True, stop=True)
                        nc.vector.reduce_max(pooled[:, i:i + 1], ps[:], axis=AX.X)
                else:
                    for k in range(R // 2):
                        h1 = h1p.tile([128, N], F32R, tag="h1")
                        nc.scalar.activation(h1[:], B2[:], AF.Relu, bias=a2[:, k:k + 1])
                        mrow = mrp.tile([65, N], F32R, tag="mr")
                        nc.sync.dma_start(mrow[0:1, :], slab[2 * k:2 * k + 1, :])
                        nc.sync.dma_start(mrow[64:65, :], slab[2 * k + 1:2 * k + 2, :])
                        pstile = {}
                        for ii, pb in ((0, 0), (1, 64)):
                            ps = psm.tile([128, N], F32, tag="ps")
                            pstile[ii] = ps
                            for bank in range(2):
                                cols = slice(bank * 512, (bank + 1) * 512)
                                nc.tensor.matmul(ps[:, cols], w2r[pb:pb + 64, :],
                                                 h1[pb:pb + 64, cols], start=True, stop=False)
                            for bank in range(2):
                                cols = slice(bank * 512, (bank + 1) * 512)
                                nc.tensor.matmul(ps[:, cols], ones[pb:pb + 1, :],
                                                 mrow[pb:pb + 1, cols], start=False, stop=True)
                        for ii in (0, 1):
                            idx = 2 * k + ii
                            nc.vector.reduce_max(pooled[:, idx:idx + 1], pstile[ii][:], axis=AX.X)

            # ---- finalize pooled + head MLP ----
            with tc.tile_pool(name="psh", bufs=1, space="PSUM") as psh:
                poolr = scr.tile([128, R], F32, tag="poolr")
                nc.scalar.activation(poolr[:], pooled[:], AF.Relu, bias=b2[:])
                poolF = scr.tile([128, R], F32, tag="poolF")
                nc.vector.tensor_scalar(poolF[:], poolr[:], h2d[:], None, op0=ALU.max)

                sgv2 = scr.tile([4, R], F32, tag="sgv2")
                nc.vector.tensor_tensor(out=sgv2[:], in0=sperm[:], in1=gT2[:], op=ALU.subtract)

                # fc1: K=132 split 64+64+4 (uniform base per group; cross-psum for base 64)
                z1a = psh.tile([64, R], F32, tag="z1a")
                nc.tensor.matmul(z1a[:], fc1aT[0:64, :], poolF[0:64, :], start=True, stop=False)
                nc.tensor.matmul(z1a[:], fc1bT[:], sgv2[:], start=False, stop=True)
                z1b = psh.tile([64, R], F32, tag="z1b")
                nc.tensor.matmul(z1b[:], fc1aT[64:128, :], poolF[64:128, :], start=True, stop=True)
                z1s = scr.tile([64, R], F32, tag="z1s")
                nc.scalar.copy(z1s[:], z1b[:])
                z1pre = scr.tile([64, R], F32, tag="z1pre")
                nc.vector.tensor_tensor(out=z1pre[:], in0=z1a[:], in1=z1s[:], op=ALU.add)
                z1 = scr.tile([64, R], F32, tag="z1")
                nc.scalar.activation(z1[:], z1pre[:], AF.Relu, bias=fb1[:])

                z2p = psh.tile([128, R], F32, tag="z2p")
                nc.tensor.matmul(z2p[:], fc2T[:], z1[:], start=True, stop=True)
                z2 = scr.tile([128, R], F32, tag="z2")
                nc.scalar.activation(z2[:], z2p[:], AF.Relu, bias=fb2[:])

                z3a = psh.tile([64, R], F32, tag="z3a")
                nc.tensor.matmul(z3a[:], fc3T[0:64, :], z2[0:64, :], start=True, stop=True)
                z3b = psh.tile([64, R], F32, tag="z3b")
                nc.tensor.matmul(z3b[:], fc3T[64:128, :], z2[64:128, :], start=True, stop=True)
                z3s = scr.tile([64, R], F32, tag="z3s")
                nc.scalar.copy(z3s[:], z3b[:])
                z3pre = scr.tile([64, R], F32, tag="z3pre")
                nc.vector.tensor_tensor(out=z3pre[:], in0=z3a[:], in1=z3s[:], op=ALU.add)
                z3 = scr.tile([64, R], F32, tag="z3")
                nc.scalar.activation(z3[:], z3pre[:], AF.Relu, bias=fb3[:])

                kp = psh.tile([4, R], F32, tag="kp")
                nc.tensor.matmul(kp[:], fc4T[:], z3[:], start=True, stop=True)
                ksig = scr.tile([4, R], F32, tag="ksig")
                nc.scalar.activation(ksig[:], kp[:], AF.Sigmoid, bias=fb4[:])
                k2n = scr.tile([4, R], F32, tag="k2n")
                nc.vector.tensor_scalar(k2n[:], ksig[:], -2.0, 1.0, op0=ALU.mult, op1=ALU.add)
                P = scr.tile([4, R], F32, tag="P")
                nc.vector.tensor_tensor(out=P[:], in0=k2n[:], in1=sgv2[:], op=ALU.mult)
                av = psh.tile([2, R], F32, tag="av")
                nc.tensor.matmul(av[:], sel4[:], P[:], start=True, stop=True)
                res = scr.tile([2, R], F32, tag="res")
                nc.scalar.copy(res[:], av[:])
                nc.sync.dma_start(out_e[:].rearrange("i o -> o i"), res[:])

    nc.compile()
    _CACHE["nc"] = nc
    return nc


def _in_maps(inputs):
    f = np.float32
    s = np.asarray(inputs["s"], f)
    g = np.asarray(inputs["g"], f)
    c1w = np.asarray(inputs["conv1_w"], f)
    c1b = np.asarray(inputs["conv1_b"], f)
    c2w = np.asarray(inputs["conv2_w"], f)
    c2b = np.asarray(inputs["conv2_b"], f)
    f1w = np.asarray(inputs["fc1_w"], f)
    f1b = np.asarray(inputs["fc1_b"], f)
    f2w = np.asarray(inputs["fc2_w"], f)
    f2b = np.asarray(inputs["fc2_b"], f)
    f3w = np.asarray(inputs["fc3_w"], f)
    f3b = np.asarray(inputs["fc3_b"], f)
    f4w = np.asarray(inputs["fc4_w"], f)
    f4b = np.asarray(inputs["fc4_b"], f)

    sT = np.ascontiguousarray(s.T)
    shared = {
        "sT": sT,
        "xrow": np.ascontiguousarray(sT[0:1]),
        "yrow": np.ascontiguousarray(sT[1:2]),
        "w1T4d": np.ascontiguousarray(np.tile(c1w[:, :4].T, (1, 2))),
        "b1dup": np.tile(c1b, 2)[:, None].astype(f),
        "c1w4": np.ascontiguousarray(c1w[:, 4:5]),
        "b1": c1b[:, None].copy(),
        "w2dup": np.ascontiguousarray(np.tile(c2w.T, (2, 1))),
        "w2K65": np.concatenate([c2w.T, np.ones((1, 128), f)], 0),
        "b2": c2b[:, None].copy(),
        "fc1aT": np.ascontiguousarray(f1w[:, :128].T),
        "fc1bT": np.ascontiguousarray(f1w[:, [128, 130, 129, 131]].T),
        "fb1": f1b[:, None].copy(),
        "fc2T": np.ascontiguousarray(f2w.T),
        "fb2": f2b[:, None].copy(),
        "fc3T": np.ascontiguousarray(f3w.T),
        "fb3": f3b[:, None].copy(),
        "fc4T": np.ascontiguousarray(f4w.T),
        "fb4": f4b[:, None].copy(),
        "sel4": np.array([[1, 0], [1, 0], [0, 1], [0, 1]], f),
    }
    eyeN = np.eye(N, dtype=f) * f(1e6)
    maps = []
    for c in range(NCORES):
        r0 = c * R
        rows = slice(r0, r0 + R)
        m = dict(shared)
        m["srows"] = s[rows].copy()
        m["strows"] = np.ascontiguousarray(s[rows].T)
        m["eye"] = eyeN[rows].copy()
        m["sperm"] = np.ascontiguousarray(s[rows][:, [0, 2, 1, 3]].T)
        gT2 = np.zeros((4, R), f)
        gT2[0] = g[rows, 0]
        gT2[2] = g[rows, 1]
        m["gT2"] = gT2
        maps.append(m)
    return maps


def kernel(**inputs) -> np.ndarray:
    from concourse.bass_utils import run_bass_kernel_spmd
    nc = _build()
    res = run_bass_kernel_spmd(nc, _in_maps(inputs), list(range(NCORES)))
    return np.concatenate([res.results[c]["out"] for c in range(NCORES)], axis=0)


# revision 13
# speedup vs baseline: 3.1207x; 1.4757x over previous
"""Trainium2 Bass kernel for nn_NetworkAction (GNN message passing).

Sharding: agents are sorted by x on the host and rows are sharded 128/core in
sorted order (locality-aware sharding).  All neighbors of a row satisfy
|x_j - x_i| < 0.5, so each core's rows only interact with one contiguous
window of W=512 sorted columns; each core receives just its window's agent
data (halo-style input distribution).  Padding columns inside the window are
genuine non-neighbors and are rejected by the on-device distance mask.
Falls back to W=1024 (full width) if any window exceeds 512.

Per core (device):
  conv1 factorizes: h1[i,j,:] = relu(u_i - u_j + b1 + delta_ij*c) with
  u = conv1_w[:, :4] @ s.T, so no [n,n,5] tensor is ever built.
  The neighbor mask (0 / -3e38) is computed exactly in fp32 elementwise
  (bitwise-matching the reference's dist<0.5 test) and rides row 64 of the
  conv2 contraction: one K=65 bf16 matmul per row gives masked h2pre in PSUM.
  pooled = relu(max_j psum + b2) (relu/bias commute with max), then max with
  the constant diagonal contribution h2diag = relu(W2 relu(b1+c) + b2).
  Head MLP: small fp32 matmuls, K split into <=64 chunks per PSUM group.
"""
import os
import sys
sys.path.insert(0, "/opt/trn_rl_repo")
import numpy as np

N = 1024
NCORES = 8
R = N // NCORES  # 128 rows per core

_CACHE = {}


def _build(W):
    key = ("nc", W)
    if key in _CACHE:
        return _CACHE[key]
    import concourse.bacc as bacc
    import concourse.mybir as mybir
    import concourse.tile as tile

    F32 = mybir.dt.float32
    BF16 = mybir.dt.bfloat16
    AX = mybir.AxisListType
    AF = mybir.ActivationFunctionType
    ALU = mybir.AluOpType

    nc = bacc.Bacc("TRN2", target_bir_lowering=False, debug=False, num_devices=NCORES)

    def din(name, shape):
        return nc.dram_tensor(name, shape, F32, kind="ExternalInput")

    sTw_e = din("sTw", [4, W])        # window agents, transposed
    xrow_e = din("xrow", [1, W])
    yrow_e = din("yrow", [1, W])
    srows_e = din("srows", [R, 4])    # this core's rows
    strows_e = din("strows", [4, R])
    eye_e = din("eye", [R, W])        # 1e6 at each row's own window column
    w1T4d_e = din("w1T4d", [4, 128])
    b1dup_e = din("b1dup", [128, 1])
    c1w4_e = din("c1w4", [64, 1])
    b1_e = din("b1", [64, 1])
    w2K65_e = din("w2K65", [65, 128])  # [conv2_w.T ; ones]
    w2T_e = din("w2T", [64, 128])
    b2_e = din("b2", [128, 1])
    fc1aT_e = din("fc1aT", [128, 64])
    fc1bT_e = din("fc1bT", [4, 64])
    fb1_e = din("fb1", [64, 1])
    fc2T_e = din("fc2T", [64, 128])
    fb2_e = din("fb2", [128, 1])
    fc3T_e = din("fc3T", [128, 64])
    fb3_e = din("fb3", [64, 1])
    fc4T_e = din("fc4T", [64, 4])
    fb4_e = din("fb4", [4, 1])
    sperm_e = din("sperm", [4, R])
    gT2_e = din("gT2", [4, R])
    sel4_e = din("sel4", [4, 2])
    out_e = nc.dram_tensor("out", [R, 2], F32, kind="ExternalOutput")

    with tile.TileContext(nc) as tc:
        with (
            tc.tile_pool(name="per", bufs=1) as per,
            tc.tile_pool(name="scr", bufs=3) as scr,
            tc.tile_pool(name="h1p", bufs=6) as h1p,
        ):
            sTw = per.tile([4, W], F32)
            xrow = per.tile([1, W], F32)
            yrow = per.tile([1, W], F32)
            srows = per.tile([R, 4], F32)
            strows = per.tile([4, R], F32)
            eye = per.tile([R, W], F32)
            w1T4d = per.tile([4, 128], F32)
            b1dup = per.tile([128, 1], F32)
            c1w4 = per.tile([64, 1], F32)
            b1 = per.tile([64, 1], F32)
            w2K65 = per.tile([65, 128], F32)
            w2T = per.tile([64, 128], F32)
            b2 = per.tile([128, 1], F32)
            fc1aT = per.tile([128, 64], F32)
            fc1bT = per.tile([4, 64], F32)
            fb1 = per.tile([64, 1], F32)
            fc2T = per.tile([64, 128], F32)
            fb2 = per.tile([128, 1], F32)
            fc3T = per.tile([128, 64], F32)
            fb3 = per.tile([64, 1], F32)
            fc4T = per.tile([64, 4], F32)
            fb4 = per.tile([4, 1], F32)
            sperm = per.tile([4, R], F32)
            gT2 = per.tile([4, R], F32)
            sel4 = per.tile([4, 2], F32)
            for t, e in ((sTw, sTw_e), (xrow, xrow_e), (yrow, yrow_e), (srows, srows_e),
                         (strows, strows_e), (eye, eye_e), (w1T4d, w1T4d_e),
                         (b1dup, b1dup_e), (c1w4, c1w4_e), (b1, b1_e),
                         (w2K65, w2K65_e), (w2T, w2T_e), (b2, b2_e),
                         (fc1aT, fc1aT_e), (fc1bT, fc1bT_e), (fb1, fb1_e),
                         (fc2T, fc2T_e), (fb2, fb2_e), (fc3T, fc3T_e), (fb3, fb3_e),
                         (fc4T, fc4T_e), (fb4, fb4_e), (sperm, sperm_e), (gT2, gT2_e),
                         (sel4, sel4_e)):
                nc.sync.dma_start(t[:], e[:])

            w2b = per.tile([65, 128], BF16)
            nc.scalar.copy(w2b[:], w2K65[:])

            B2 = per.tile([64, W], F32)
            a_all = per.tile([128, R], F32)
            h2d = per.tile([128, 1], F32)
            slab = per.tile([R, W], BF16)
            pooled = per.tile([128, R], F32)

            with tc.tile_pool(name="pss", bufs=1, space="PSUM") as pss:
                # u over window agents -> B2 = -u (lhsT kept full [4,128])
                uw = pss.tile([128, W], F32, tag="uw")
                for b0 in range(0, W, 512):
                    nc.tensor.matmul(uw[:, b0:b0 + 512], w1T4d[:],
                                     sTw[:, b0:b0 + 512], start=True, stop=True)
                nc.scalar.activation(B2[:], uw[0:64, :], AF.Copy, scale=-1.0)

                # a_all = u(rows) + b1 (duplicated on both halves)
                u2r = pss.tile([128, R], F32, tag="u2r")
                nc.tensor.matmul(u2r[:], w1T4d[:], strows[:], start=True, stop=True)
                nc.scalar.activation(a_all[:], u2r[:], AF.Identity, bias=b1dup[:])

                # h2diag = relu(W2 @ relu(c1w4 + b1) + b2)
                h1d = scr.tile([64, 1], F32, tag="h1d")
                nc.scalar.activation(h1d[:], c1w4[:], AF.Relu, bias=b1[:])
                psd = pss.tile([128, 1], F32, tag="psd")
                nc.tensor.matmul(psd[:], w2T[:], h1d[:], start=True, stop=True)
                nc.scalar.activation(h2d[:], psd[:], AF.Relu, bias=b2[:])

            # mask slab: (d2 + 1e6*eye >= 0.25) * -3e38  (d2 exact fp32)
            pbx = scr.tile([R, W], F32, tag="big")
            nc.gpsimd.partition_broadcast(pbx[:], xrow[:])
            pdx = scr.tile([R, W], F32, tag="big")
            nc.vector.tensor_scalar(pdx[:], pbx[:], srows[:, 0:1], None, op0=ALU.subtract)
            sqx = scr.tile([R, W], F32, tag="big")
            nc.scalar.square(sqx[:], pdx[:])
            pby = scr.tile([R, W], F32, tag="big2")
            nc.gpsimd.partition_broadcast(pby[:], yrow[:])
            pdy = scr.tile([R, W], F32, tag="big2")
            nc.vector.tensor_scalar(pdy[:], pby[:], srows[:, 1:2], None, op0=ALU.subtract)
            sqy = scr.tile([R, W], F32, tag="big2")
            nc.scalar.square(sqy[:], pdy[:])
            d2a = scr.tile([R, W], F32, tag="big")
            nc.vector.tensor_tensor(out=d2a[:], in0=sqx[:], in1=sqy[:], op=ALU.add)
            d2p = scr.tile([R, W], F32, tag="big2")
            nc.vector.tensor_tensor(out=d2p[:], in0=d2a[:], in1=eye[:], op=ALU.add)
            nc.vector.tensor_scalar(slab[:], d2p[:], 0.25, -3.0e38, op0=ALU.is_ge, op1=ALU.mult)

            # ---- main loop: per row i one K=65 bf16 matmul per 512-col bank ----
            nbanks = W // 512
            with tc.tile_pool(name="psm", bufs=6, space="PSUM") as psm:
                for i in range(R):
                    h1t = h1p.tile([65, W], BF16, tag="h1")
                    nc.scalar.activation(h1t[0:64, :], B2[:], AF.Relu,
                                         bias=a_all[0:64, i:i + 1])
                    nc.sync.dma_start(h1t[64:65, :], slab[i:i + 1, :])
                    ps = psm.tile([128, W], F32, tag="ps")
                    for bk in range(nbanks):
                        cols = slice(bk * 512, (bk + 1) * 512)
                        nc.tensor.matmul(ps[:, cols], w2b[:], h1t[:, cols],
                                         start=True, stop=True)
                    nc.vector.reduce_max(pooled[:, i:i + 1], ps[:], axis=AX.X)

            # ---- finalize pooled + head MLP ----
            with tc.tile_pool(name="psh", bufs=1, space="PSUM") as psh:
                poolr = scr.tile([128, R], F32, tag="poolr")
                nc.scalar.activation(poolr[:], pooled[:], AF.Relu, bias=b2[:])
                poolF = scr.tile([128, R], F32, tag="poolF")
                nc.vector.tensor_scalar(poolF[:], poolr[:], h2d[:], None, op0=ALU.max)

                sgv2 = scr.tile([4, R], F32, tag="sgv2")
                nc.vector.tensor_tensor(out=sgv2[:], in0=sperm[:], in1=gT2[:], op=ALU.subtract)

                z1a = psh.tile([64, R], F32, tag="z1a")
                nc.tensor.matmul(z1a[:], fc1aT[0:64, :], poolF[0:64, :], start=True, stop=False)
                nc.tensor.matmul(z1a[:], fc1bT[:], sgv2[:], start=False, stop=True)
                z1b = psh.tile([64, R], F32, tag="z1b")
                nc.tensor.matmul(z1b[:], fc1aT[64:128, :], poolF[64:128, :], start=True, stop=True)
                z1s = scr.tile([64, R], F32, tag="z1s")
                nc.scalar.copy(z1s[:], z1b[:])
                z1pre = scr.tile([64, R], F32, tag="z1pre")
                nc.vector.tensor_tensor(out=z1pre[:], in0=z1a[:], in1=z1s[:], op=ALU.add)
                z1 = scr.tile([64, R], F32, tag="z1")
                nc.scalar.activation(z1[:], z1pre[:], AF.Relu, bias=fb1[:])

                z2p = psh.tile([128, R], F32, tag="z2p")
                nc.tensor.matmul(z2p[:], fc2T[:], z1[:], start=True, stop=True)
                z2 = scr.tile([128, R], F32, tag="z2")
                nc.scalar.activation(z2[:], z2p[:], AF.Relu, bias=fb2[:])

                z3a = psh.tile([64, R], F32, tag="z3a")
                nc.tensor.matmul(z3a[:], fc3T[0:64, :], z2[0:64, :], start=True, stop=True)
                z3b = psh.tile([64, R], F32, tag="z3b")
                nc.tensor.matmul(z3b[:], fc3T[64:128, :], z2[64:128, :], start=True, stop=True)
                z3s = scr.tile([64, R], F32, tag="z3s")
                nc.scalar.copy(z3s[:], z3b[:])
                z3pre = scr.tile([64, R], F32, tag="z3pre")
                nc.vector.tensor_tensor(out=z3pre[:], in0=z3a[:], in1=z3s[:], op=ALU.add)
                z3 = scr.tile([64, R], F32, tag="z3")
                nc.scalar.activation(z3[:], z3pre[:], AF.Relu, bias=fb3[:])

                kp = psh.tile([4, R], F32, tag="kp")
                nc.tensor.matmul(kp[:], fc4T[:], z3[:], start=True, stop=True)
                ksig = scr.tile([4, R], F32, tag="ksig")
                nc.scalar.activation(ksig[:], kp[:], AF.Sigmoid, bias=fb4[:])
                k2n = scr.tile([4, R], F32, tag="k2n")
                nc.vector.tensor_scalar(k2n[:], ksig[:], -2.0, 1.0, op0=ALU.mult, op1=ALU.add)
                P = scr.tile([4, R], F32, tag="P")
                nc.vector.tensor_tensor(out=P[:], in0=k2n[:], in1=sgv2[:], op=ALU.mult)
                av = psh.tile([2, R], F32, tag="av")
                nc.tensor.matmul(av[:], sel4[:], P[:], start=True, stop=True)
                res = scr.tile([2, R], F32, tag="res")
                nc.scalar.copy(res[:], av[:])
                nc.sync.dma_start(out_e[:].rearrange("i o -> o i"), res[:])

    nc.compile()
    _CACHE[key] = nc
    return nc


def _prep(inputs):
    """Sort agents by x; compute per-core windows. Returns (W, perm, windows)."""
    s = np.asarray(inputs["s"], np.float32)
    perm = np.argsort(s[:, 0], kind="stable")
    xs = s[perm, 0].astype(np.float64)
    los, his = [], []
    for c in range(NCORES):
        rx = xs[c * R:(c + 1) * R]
        lo = int(np.searchsorted(xs, rx.min() - 0.5, "left"))
        hi = int(np.searchsorted(xs, rx.max() + 0.5, "right"))
        los.append(lo)
        his.append(hi)
    maxw = max(h - l for l, h in zip(los, his))
    if maxw <= 512:
        W = 512
    else:
        W = N
        perm = np.arange(N)
        los = [0] * NCORES
    # clamp window starts so [lo, lo+W) stays in range and covers [lo, hi)
    starts = [min(max(0, l), N - W) for l in los]
    return W, perm, starts


def _in_maps(inputs):
    f = np.float32
    s = np.asarray(inputs["s"], f)
    g = np.asarray(inputs["g"], f)
    c1w = np.asarray(inputs["conv1_w"], f)
    c1b = np.asarray(inputs["conv1_b"], f)
    c2w = np.asarray(inputs["conv2_w"], f)
    c2b = np.asarray(inputs["conv2_b"], f)
    f1w = np.asarray(inputs["fc1_w"], f)
    f1b = np.asarray(inputs["fc1_b"], f)
    f2w = np.asarray(inputs["fc2_w"], f)
    f2b = np.asarray(inputs["fc2_b"], f)
    f3w = np.asarray(inputs["fc3_w"], f)
    f3b = np.asarray(inputs["fc3_b"], f)
    f4w = np.asarray(inputs["fc4_w"], f)
    f4b = np.asarray(inputs["fc4_b"], f)

    W, perm, starts = _prep(inputs)
    sp = s[perm]
    gp = g[perm]
    shared = {
        "w1T4d": np.ascontiguousarray(np.tile(c1w[:, :4].T, (1, 2))),
        "b1dup": np.tile(c1b, 2)[:, None].astype(f),
        "c1w4": np.ascontiguousarray(c1w[:, 4:5]),
        "b1": c1b[:, None].copy(),
        "w2K65": np.concatenate([c2w.T, np.ones((1, 128), f)], 0),
        "w2T": np.ascontiguousarray(c2w.T),
        "b2": c2b[:, None].copy(),
        "fc1aT": np.ascontiguousarray(f1w[:, :128].T),
        "fc1bT": np.ascontiguousarray(f1w[:, [128, 130, 129, 131]].T),
        "fb1": f1b[:, None].copy(),
        "fc2T": np.ascontiguousarray(f2w.T),
        "fb2": f2b[:, None].copy(),
        "fc3T": np.ascontiguousarray(f3w.T),
        "fb3": f3b[:, None].copy(),
        "fc4T": np.ascontiguousarray(f4w.T),
        "fb4": f4b[:, None].copy(),
        "sel4": np.array([[1, 0], [1, 0], [0, 1], [0, 1]], f),
    }
    maps = []
    for c in range(NCORES):
        rows = slice(c * R, (c + 1) * R)
        w0 = starts[c]
        win = slice(w0, w0 + W)
        m = dict(shared)
        sw = sp[win]
        m["sTw"] = np.ascontiguousarray(sw.T)
        m["xrow"] = np.ascontiguousarray(sw.T[0:1])
        m["yrow"] = np.ascontiguousarray(sw.T[1:2])
        m["srows"] = sp[rows].copy()
        m["strows"] = np.ascontiguousarray(sp[rows].T)
        eye = np.zeros((R, W), f)
        for il in range(R):
            eye[il, c * R + il - w0] = f(1e6)
        m["eye"] = eye
        m["sperm"] = np.ascontiguousarray(sp[rows][:, [0, 2, 1, 3]].T)
        gT2 = np.zeros((4, R), f)
        gT2[0] = gp[rows, 0]
        gT2[2] = gp[rows, 1]
        m["gT2"] = gT2
        maps.append(m)
    return maps, W, perm


def kernel(**inputs) -> np.ndarray:
    from concourse.bass_utils import run_bass_kernel_spmd
    maps, W, perm = _in_maps(inputs)
    nc = _build(W)
    res = run_bass_kernel_spmd(nc, maps, list(range(NCORES)))
    out_sorted = np.concatenate([res.results[c]["out"] for c in range(NCORES)], axis=0)
    out = np.empty_like(out_sorted)
    out[perm] = out_sorted
    return out


# revision 15
# speedup vs baseline: 3.3909x; 1.0866x over previous
"""Trainium2 Bass kernel for nn_NetworkAction (GNN message passing).

Sharding: agents are sorted by x on the host and rows are sharded 128/core in
sorted order (locality-aware sharding).  All neighbors of a row satisfy
|x_j - x_i| < 0.5, so each core's rows only interact with one contiguous
window of W=512 sorted columns; each core receives just its window's agent
data (halo-style input distribution).  Padding columns inside the window are
genuine non-neighbors and are rejected by the on-device distance mask.
Falls back to W=1024 (full width) if any window exceeds 512.

Per core (device):
  conv1 factorizes: h1[i,j,:] = relu(u_i - u_j + b1 + delta_ij*c) with
  u = conv1_w[:, :4] @ s.T, so no [n,n,5] tensor is ever built.
  The neighbor mask (0 / -3e38) is computed exactly in fp32 elementwise
  (bitwise-matching the reference's dist<0.5 test) and rides row 64 of the
  conv2 contraction: one K=65 bf16 matmul per row gives masked h2pre in PSUM.
  pooled = relu(max_j psum + b2) (relu/bias commute with max), then max with
  the constant diagonal contribution h2diag = relu(W2 relu(b1+c) + b2).
  Head MLP: small fp32 matmuls, K split into <=64 chunks per PSUM group.
"""
import os
import sys
sys.path.insert(0, "/opt/trn_rl_repo")
import numpy as np

N = 1024
NCORES = 8
R = N // NCORES  # 128 rows per core

_CACHE = {}


def _build(W):
    key = ("nc", W)
    if key in _CACHE:
        return _CACHE[key]
    import concourse.bacc as bacc
    import concourse.mybir as mybir
    import concourse.tile as tile

    F32 = mybir.dt.float32
    BF16 = mybir.dt.bfloat16
    AX = mybir.AxisListType
    AF = mybir.ActivationFunctionType
    ALU = mybir.AluOpType

    nc = bacc.Bacc("TRN2", target_bir_lowering=False, debug=False, num_devices=NCORES)

    def din(name, shape):
        return nc.dram_tensor(name, shape, F32, kind="ExternalInput")

    sTw_e = din("sTw", [4, W])        # window agents, transposed
    xrow_e = din("xrow", [1, W])
    yrow_e = din("yrow", [1, W])
    srows_e = din("srows", [R, 4])    # this core's rows
    strows_e = din("strows", [4, R])
    eye_e = din("eye", [R, W])        # 1e6 at each row's own window column
    w1T4d_e = din("w1T4d", [4, 128])
    b1dup_e = din("b1dup", [128, 1])
    c1w4_e = din("c1w4", [64, 1])
    b1_e = din("b1", [64, 1])
    w2K65_e = din("w2K65", [65, 128])  # [conv2_w.T ; ones]
    w2T_e = din("w2T", [64, 128])
    b2_e = din("b2", [128, 1])
    fc1aT_e = din("fc1aT", [128, 64])
    fc1bT_e = din("fc1bT", [4, 64])
    fb1_e = din("fb1", [64, 1])
    fc2T_e = din("fc2T", [64, 128])
    fb2_e = din("fb2", [128, 1])
    fc3T_e = din("fc3T", [128, 64])
    fb3_e = din("fb3", [64, 1])
    fc4T_e = din("fc4T", [64, 4])
    fb4_e = din("fb4", [4, 1])
    sperm_e = din("sperm", [4, R])
    gT2_e = din("gT2", [4, R])
    sel4_e = din("sel4", [4, 2])
    out_e = nc.dram_tensor("out", [R, 2], F32, kind="ExternalOutput")

    with tile.TileContext(nc) as tc:
        with (
            tc.tile_pool(name="per", bufs=1) as per,
            tc.tile_pool(name="scr", bufs=3) as scr,
            tc.tile_pool(name="h1p", bufs=3) as h1p,
        ):
            sTw = per.tile([4, W], F32)
            xrow = per.tile([1, W], F32)
            yrow = per.tile([1, W], F32)
            srows = per.tile([R, 4], F32)
            strows = per.tile([4, R], F32)
            eye = per.tile([R, W], F32)
            w1T4d = per.tile([4, 128], F32)
            b1dup = per.tile([128, 1], F32)
            c1w4 = per.tile([64, 1], F32)
            b1 = per.tile([64, 1], F32)
            w2K65 = per.tile([65, 128], F32)
            w2T = per.tile([64, 128], F32)
            b2 = per.tile([128, 1], F32)
            fc1aT = per.tile([128, 64], F32)
            fc1bT = per.tile([4, 64], F32)
            fb1 = per.tile([64, 1], F32)
            fc2T = per.tile([64, 128], F32)
            fb2 = per.tile([128, 1], F32)
            fc3T = per.tile([128, 64], F32)
            fb3 = per.tile([64, 1], F32)
            fc4T = per.tile([64, 4], F32)
            fb4 = per.tile([4, 1], F32)
            sperm = per.tile([4, R], F32)
            gT2 = per.tile([4, R], F32)
            sel4 = per.tile([4, 2], F32)
            for t, e in ((sTw, sTw_e), (xrow, xrow_e), (yrow, yrow_e), (srows, srows_e),
                         (strows, strows_e), (eye, eye_e), (w1T4d, w1T4d_e),
                         (b1dup, b1dup_e), (c1w4, c1w4_e), (b1, b1_e),
                         (w2K65, w2K65_e), (w2T, w2T_e), (b2, b2_e),
                         (fc1aT, fc1aT_e), (fc1bT, fc1bT_e), (fb1, fb1_e),
                         (fc2T, fc2T_e), (fb2, fb2_e), (fc3T, fc3T_e), (fb3, fb3_e),
                         (fc4T, fc4T_e), (fb4, fb4_e), (sperm, sperm_e), (gT2, gT2_e),
                         (sel4, sel4_e)):
                nc.sync.dma_start(t[:], e[:])

            w2b = per.tile([65, 128], BF16)
            nc.scalar.copy(w2b[:], w2K65[:])

            B2 = per.tile([64, W], F32)
            a_all = per.tile([128, R], F32)
            h2d = per.tile([128, 1], F32)
            slab = per.tile([R, W], BF16)
            pooled = per.tile([128, R], F32)

            with tc.tile_pool(name="pss", bufs=1, space="PSUM") as pss:
                # u over window agents -> B2 = -u (lhsT kept full [4,128])
                uw = pss.tile([128, W], F32, tag="uw")
                for b0 in range(0, W, 512):
                    nc.tensor.matmul(uw[:, b0:b0 + 512], w1T4d[:],
                                     sTw[:, b0:b0 + 512], start=True, stop=True)
                nc.scalar.activation(B2[:], uw[0:64, :], AF.Copy, scale=-1.0)

                # a_all = u(rows) + b1 (duplicated on both halves)
                u2r = pss.tile([128, R], F32, tag="u2r")
                nc.tensor.matmul(u2r[:], w1T4d[:], strows[:], start=True, stop=True)
                nc.scalar.activation(a_all[:], u2r[:], AF.Identity, bias=b1dup[:])

                # h2diag = relu(W2 @ relu(c1w4 + b1) + b2)
                h1d = scr.tile([64, 1], F32, tag="h1d")
                nc.scalar.activation(h1d[:], c1w4[:], AF.Relu, bias=b1[:])
                psd = pss.tile([128, 1], F32, tag="psd")
                nc.tensor.matmul(psd[:], w2T[:], h1d[:], start=True, stop=True)
                nc.scalar.activation(h2d[:], psd[:], AF.Relu, bias=b2[:])

            # mask slab: (d2 + 1e6*eye >= 0.25) * -3e38  (d2 exact fp32)
            pbx = scr.tile([R, W], F32, tag="big")
            nc.gpsimd.partition_broadcast(pbx[:], xrow[:])
            pdx = scr.tile([R, W], F32, tag="big")
            nc.vector.tensor_scalar(pdx[:], pbx[:], srows[:, 0:1], None, op0=ALU.subtract)
            sqx = scr.tile([R, W], F32, tag="big")
            nc.scalar.square(sqx[:], pdx[:])
            pby = scr.tile([R, W], F32, tag="big2")
            nc.gpsimd.partition_broadcast(pby[:], yrow[:])
            pdy = scr.tile([R, W], F32, tag="big2")
            nc.vector.tensor_scalar(pdy[:], pby[:], srows[:, 1:2], None, op0=ALU.subtract)
            sqy = scr.tile([R, W], F32, tag="big2")
            nc.scalar.square(sqy[:], pdy[:])
            d2a = scr.tile([R, W], F32, tag="big")
            nc.vector.tensor_tensor(out=d2a[:], in0=sqx[:], in1=sqy[:], op=ALU.add)
            d2p = scr.tile([R, W], F32, tag="big2")
            nc.vector.tensor_tensor(out=d2p[:], in0=d2a[:], in1=eye[:], op=ALU.add)
            nc.vector.tensor_scalar(slab[:], d2p[:], 0.25, -3.0e38, op0=ALU.is_ge, op1=ALU.mult)

            # ---- main loop ----
            # Rows processed in blocks of 8: one [65, 8W] tile holds 8 rows'
            # h1 (+ their mask rows staged by a single DMA into partition 64),
            # so each matmul is one K=65 bf16 MM and waits only on its ACT
            # build; per-row one reduce_max drains the PSUM bank.
            BLK = 8
            nbanks = W // 512
            psm_bufs = 6 if nbanks == 1 else 4
            with tc.tile_pool(name="psm", bufs=psm_bufs, space="PSUM") as psm:
                for b in range(R // BLK):
                    m64 = h1p.tile([65, BLK * W], BF16, tag="h1")
                    nc.sync.dma_start(m64[64:65, :], slab[b * BLK:(b + 1) * BLK, :])
                    for t in range(BLK):
                        i = b * BLK + t
                        off = t * W
                        nc.scalar.activation(m64[0:64, off:off + W], B2[:], AF.Relu,
                                             bias=a_all[0:64, i:i + 1])
                        ps = psm.tile([128, W], F32, tag="ps")
                        for bk in range(nbanks):
                            cols = slice(bk * 512, (bk + 1) * 512)
                            nc.tensor.matmul(ps[:, cols], w2b[:],
                                             m64[:, off + bk * 512:off + (bk + 1) * 512],
                                             start=True, stop=True)
                        nc.vector.reduce_max(pooled[:, i:i + 1], ps[:], axis=AX.X)

            # ---- finalize pooled + head MLP ----
            with tc.tile_pool(name="psh", bufs=1, space="PSUM") as psh:
                poolr = scr.tile([128, R], F32, tag="poolr")
                nc.scalar.activation(poolr[:], pooled[:], AF.Relu, bias=b2[:])
                poolF = scr.tile([128, R], F32, tag="poolF")
                nc.vector.tensor_scalar(poolF[:], poolr[:], h2d[:], None, op0=ALU.max)

                sgv2 = scr.tile([4, R], F32, tag="sgv2")
                nc.vector.tensor_tensor(out=sgv2[:], in0=sperm[:], in1=gT2[:], op=ALU.subtract)

                z1a = psh.tile([64, R], F32, tag="z1a")
                nc.tensor.matmul(z1a[:], fc1aT[0:64, :], poolF[0:64, :], start=True, stop=False)
                nc.tensor.matmul(z1a[:], fc1bT[:], sgv2[:], start=False, stop=True)
                z1b = psh.tile([64, R], F32, tag="z1b")
                nc.tensor.matmul(z1b[:], fc1aT[64:128, :], poolF[64:128, :], start=True, stop=True)
                z1s = scr.tile([64, R], F32, tag="z1s")
                nc.scalar.copy(z1s[:], z1b[:])
                z1pre = scr.tile([64, R], F32, tag="z1pre")
                nc.vector.tensor_tensor(out=z1pre[:], in0=z1a[:], in1=z1s[:], op=ALU.add)
                z1 = scr.tile([64, R], F32, tag="z1")
                nc.scalar.activation(z1[:], z1pre[:], AF.Relu, bias=fb1[:])

                z2p = psh.tile([128, R], F32, tag="z2p")
                nc.tensor.matmul(z2p[:], fc2T[:], z1[:], start=True, stop=True)
                z2 = scr.tile([128, R], F32, tag="z2")
                nc.scalar.activation(z2[:], z2p[:], AF.Relu, bias=fb2[:])

                z3a = psh.tile([64, R], F32, tag="z3a")
                nc.tensor.matmul(z3a[:], fc3T[0:64, :], z2[0:64, :], start=True, stop=True)
                z3b = psh.tile([64, R], F32, tag="z3b")
                nc.tensor.matmul(z3b[:], fc3T[64:128, :], z2[64:128, :], start=True, stop=True)
                z3s = scr.tile([64, R], F32, tag="z3s")
                nc.scalar.copy(z3s[:], z3b[:])
                z3pre = scr.tile([64, R], F32, tag="z3pre")
                nc.vector.tensor_tensor(out=z3pre[:], in0=z3a[:], in1=z3s[:], op=ALU.add)
                z3 = scr.tile([64, R], F32, tag="z3")
                nc.scalar.activation(z3[:], z3pre[:], AF.Relu, bias=fb3[:])

                kp = psh.tile([4, R], F32, tag="kp")
                nc.tensor.matmul(kp[:], fc4T[:], z3[:], start=True, stop=True)
                ksig = scr.tile([4, R], F32, tag="ksig")
                nc.scalar.activation(ksig[:], kp[:], AF.Sigmoid, bias=fb4[:])
                k2n = scr.tile([4, R], F32, tag="k2n")
                nc.vector.tensor_scalar(k2n[:], ksig[:], -2.0, 1.0, op0=ALU.mult, op1=ALU.add)
                P = scr.tile([4, R], F32, tag="P")
                nc.vector.tensor_tensor(out=P[:], in0=k2n[:], in1=sgv2[:], op=ALU.mult)
                av = psh.tile([2, R], F32, tag="av")
                nc.tensor.matmul(av[:], sel4[:], P[:], start=True, stop=True)
                res = scr.tile([2, R], F32, tag="res")
                nc.scalar.copy(res[:], av[:])
                nc.sync.dma_start(out_e[:].rearrange("i o -> o i"), res[:])

    nc.compile()
    _CACHE[key] = nc
    return nc


def _prep(inputs):
    """Sort agents by x; compute per-core windows. Returns (W, perm, windows)."""
    s = np.asarray(inputs["s"], np.float32)
    perm = np.argsort(s[:, 0], kind="stable")
    xs = s[perm, 0].astype(np.float64)
    los, his = [], []
    for c in range(NCORES):
        rx = xs[c * R:(c + 1) * R]
        lo = int(np.searchsorted(xs, rx.min() - 0.5, "left"))
        hi = int(np.searchsorted(xs, rx.max() + 0.5, "right"))
        los.append(lo)
        his.append(hi)
    maxw = max(h - l for l, h in zip(los, his))
    if maxw <= 512:
        W = 512
    else:
        W = N
        perm = np.arange(N)
        los = [0] * NCORES
    # clamp window starts so [lo, lo+W) stays in range and covers [lo, hi)
    starts = [min(max(0, l), N - W) for l in los]
    return W, perm, starts


def _in_maps(inputs):
    f = np.float32
    s = np.asarray(inputs["s"], f)
    g = np.asarray(inputs["g"], f)
    c1w = np.asarray(inputs["conv1_w"], f)
    c1b = np.asarray(inputs["conv1_b"], f)
    c2w = np.asarray(inputs["conv2_w"], f)
    c2b = np.asarray(inputs["conv2_b"], f)
    f1w = np.asarray(inputs["fc1_w"], f)
    f1b = np.asarray(inputs["fc1_b"], f)
    f2w = np.asarray(inputs["fc2_w"], f)
    f2b = np.asarray(inputs["fc2_b"], f)
    f3w = np.asarray(inputs["fc3_w"], f)
    f3b = np.asarray(inputs["fc3_b"], f)
    f4w = np.asarray(inputs["fc4_w"], f)
    f4b = np.asarray(inputs["fc4_b"], f)

    W, perm, starts = _prep(inputs)
    sp = s[perm]
    gp = g[perm]
    shared = {
        "w1T4d": np.ascontiguousarray(np.tile(c1w[:, :4].T, (1, 2))),
        "b1dup": np.tile(c1b, 2)[:, None].astype(f),
        "c1w4": np.ascontiguousarray(c1w[:, 4:5]),
        "b1": c1b[:, None].copy(),
        "w2K65": np.concatenate([c2w.T, np.ones((1, 128), f)], 0),
        "w2T": np.ascontiguousarray(c2w.T),
        "b2": c2b[:, None].copy(),
        "fc1aT": np.ascontiguousarray(f1w[:, :128].T),
        "fc1bT": np.ascontiguousarray(f1w[:, [128, 130, 129, 131]].T),
        "fb1": f1b[:, None].copy(),
        "fc2T": np.ascontiguousarray(f2w.T),
        "fb2": f2b[:, None].copy(),
        "fc3T": np.ascontiguousarray(f3w.T),
        "fb3": f3b[:, None].copy(),
        "fc4T": np.ascontiguousarray(f4w.T),
        "fb4": f4b[:, None].copy(),
        "sel4": np.array([[1, 0], [1, 0], [0, 1], [0, 1]], f),
    }
    maps = []
    for c in range(NCORES):
        rows = slice(c * R, (c + 1) * R)
        w0 = starts[c]
        win = slice(w0, w0 + W)
        m = dict(shared)
        sw = sp[win]
        m["sTw"] = np.ascontiguousarray(sw.T)
        m["xrow"] = np.ascontiguousarray(sw.T[0:1])
        m["yrow"] = np.ascontiguousarray(sw.T[1:2])
        m["srows"] = sp[rows].copy()
        m["strows"] = np.ascontiguousarray(sp[rows].T)
        eye = np.zeros((R, W), f)
        for il in range(R):
            eye[il, c * R + il - w0] = f(1e6)
        m["eye"] = eye
        m["sperm"] = np.ascontiguousarray(sp[rows][:, [0, 2, 1, 3]].T)
        gT2 = np.zeros((4, R), f)
        gT2[0] = gp[rows, 0]
        gT2[2] = gp[rows, 1]
        m["gT2"] = gT2
        maps.append(m)
    return maps, W, perm


def kernel(**inputs) -> np.ndarray:
    from concourse.bass_utils import run_bass_kernel_spmd
    maps, W, perm = _in_maps(inputs)
    nc = _build(W)
    res = run_bass_kernel_spmd(nc, maps, list(range(NCORES)))
    out_sorted = np.concatenate([res.results[c]["out"] for c in range(NCORES)], axis=0)
    out = np.empty_like(out_sorted)
    out[perm] = out_sorted
    return out
